# revision 9
# baseline (speedup 1.0000x reference)
"""Trainium2 Bass kernel for nn_MAB: MHA block (B=4, N=2048, D=256, H=8) on 8 cores.

Sharding: 8 shards = (batch b, query-half) pairs. Each core computes the full
attention + LN/FFN tail for its 1024 query rows against all 2048 keys of its
batch. All gathering happens on host; no collectives.

v2 structure (vs baseline):
- softmax exp is split across TWO engines: ACT (exact exp, scale=64) and DVE
  (custom fused op computing (1+x/64)^64 ~= exp(x), 7 ALU blocks, 1 elem/cyc).
  The x/64 pre-scale is folded into Wk on the host.
- PE stream is software-pipelined: QK(kt) is emitted two steps before PV(kt)
  so the tensor engine never waits on the exp engines (sc PSUM pool bufs=3).
- PSUM evictions with per-partition bias ride the Scalar engine (Identity /
  Relu / Copy live in the same ACT table as Exp -> no table reloads at all).
- LN rstd uses fixed-seed Newton iterations in custom DVE ops (no Sqrt table).
- LN gamma/beta applies, softmax divides and the residual add run on the
  otherwise-idle GpSimd engine.
- fc_o runs in fp16 (Wo cast on host); LN0/fc/residual chain keeps fp16
  tensors so PE transposes cost 1 cycle/row.
"""

import numpy as np

import concourse.bass as bass
import concourse.tile as tile
from concourse import bacc, mybir
from concourse import bass_utils
from concourse import dve_ops
from concourse.dve_spec import (
    C0, C1, C2, Spec, Src0, Src1, One, sq, lower, spec_leaves,
)
from concourse.dve_uop import DveOpSpec
from concourse.masks import make_identity

B, NQ, NK, DV, H = 4, 2048, 2048, 256, 8
HD = DV // H  # 32
NQC = 1024  # q rows per core
SCALE = 1.0 / np.sqrt(HD)
EXP_N = 64.0  # (1+x/N)^N exp approximation on DVE
EPS = 1e-5
FP16 = mybir.dt.float16
FP32 = mybir.dt.float32

# kt tiles (of 16 per head-pair) whose exp runs on ACT; rest on DVE.
ACT_KT = frozenset({0, 1, 2, 4, 6, 8, 10, 12, 14})

# Newton rsqrt seeds (only need to be within ~50% of 1/sqrt(var+eps)).
RSQRT_SEED_LN0 = 130.0   # var(o/denom) ~ 5e-5 (+eps 1e-5)
RSQRT_SEED_LN1 = 0.95    # var(ln0 + relu(fc)) ~ 1.1


def _register():
    """Register custom DVE ops (exp approx + Newton rsqrt); shas are
    computed at import so they always match the lowering."""
    ver = "v3"  # TRN2
    made = {}

    def add(name, spec):
        existing = {o.name: o for o in dve_ops.OPS}
        if name in existing:
            made[name] = existing[name]
            return
        row = max(dve_ops._SUB_OPCODE_FOR_NAME.values()) + 1
        assert row < 0x20
        dve_ops._SUB_OPCODE_FOR_NAME[name] = row
        uops = lower(spec, ver=ver)
        sha = DveOpSpec(
            name=name, opcode=row, uops=uops, rd1_en=Src1 in spec_leaves(spec)
        ).sha(ver)
        op = dve_ops.DveOp(name, spec, subdim=False, uops_sha={ver: sha})
        dve_ops.OPS.append(op)
        dve_ops.CUSTOM_DVE_SPECS[name] = spec
        made[name] = op

    # out = (1 + x)^64  (input pre-scaled by 1/64; approximates exp(64x))
    add("EXP_POW64_ANT", Spec(
        body=sq(sq(sq(sq(sq(sq(One + Src0)))))),
        reference=lambda in0, in1, s0, s1, imm2:
            ((1.0 + in0.astype(np.float64)) ** 64).astype(np.float32),
    ))

    # Two Newton rsqrt iterations from a compile-time seed y0:
    #   y1 = C0 - v*C1                 (C0 = 1.5*y0, C1 = 0.5*y0^3)
    #   out = y1*((1+C2) - (v*y1^2)*C2)  (C2 = 0.5)
    y1 = C0 - Src0 * C1
    add("RSQRT_NEWTON2_ANT", Spec(
        body=y1 * ((One + C2) - (Src0 * sq(y1)) * C2),
        reference=lambda in0, in1, s0, s1, imm2: (
            lambda v, yy: (yy * (1.5 - 0.5 * v * yy * yy))
        )(in0.astype(np.float64),
          s0 - in0.astype(np.float64) * s1).astype(np.float32),
    ))

    # One more Newton step: out = y*((1+C0) - (v*y^2)*C0)  (C0 = 0.5)
    add("RSQRT_NEWTON1_ANT", Spec(
        body=Src1 * ((One + C0) - (Src0 * sq(Src1)) * C0),
        reference=lambda in0, in1, s0, s1, imm2:
            (in1.astype(np.float64) * (1.5 - 0.5 * in0.astype(np.float64)
             * in1.astype(np.float64) ** 2)).astype(np.float32),
    ))
    return made


_DVE_OPS = _register()
EXP64 = _DVE_OPS["EXP_POW64_ANT"]
NEWTON2 = _DVE_OPS["RSQRT_NEWTON2_ANT"]
NEWTON1 = _DVE_OPS["RSQRT_NEWTON1_ANT"]


def _build():
    nc = bacc.Bacc(
        "TRN2",
        target_bir_lowering=False,
        debug=False,
        enable_asserts=False,
        num_devices=1,
    )
    d = {}
    ins = [
        ("qt", [128, 2, NQC], FP16),       # Q-shard^T  [dq(part), dq-chunk, q]
        ("kt", [128, 2, NK], FP16),        # K^T        [dq(part), dq-chunk, k]
        ("wq", [128, 2, 256], FP16),       # Wq^T       [dq(part), dq-chunk, dv]
        ("wk", [128, 2, 256], FP16),       # pre-scaled by SCALE/EXP_N
        ("wv", [128, 2, 256], FP16),
        ("wo", [128, 2, 256], FP16),       # Wo^T
        ("bq", [128, 2], FP32),            # per-dv-channel biases, chunk-major
        ("bk", [128, 2], FP32),
        ("bo", [128, 2], FP32),
        ("bvr", [128, 256], FP32),         # bv replicated over partitions
        ("g0r", [128, 256], FP32),
        ("b0r", [128, 256], FP32),
        ("g1r", [128, 256], FP32),
        ("b1r", [128, 256], FP32),
    ]
    for name, shape, dt in ins:
        d[name] = nc.dram_tensor(name, shape, dt, kind="ExternalInput").ap()
    out_dram = nc.dram_tensor("out", [NQC, 256], FP32, kind="ExternalOutput").ap()

    with tile.TileContext(nc) as tc:
        _kernel_body(tc, d, out_dram)
    nc.compile()
    return nc


def _kernel_body(tc, d, out_dram):
    nc = tc.nc
    from contextlib import ExitStack

    ctx = ExitStack()
    with ctx:
        singles = ctx.enter_context(tc.tile_pool(name="singles", bufs=1))
        small = ctx.enter_context(tc.tile_pool(name="small", bufs=8))

        # ---- load constants / inputs to SBUF ----
        sb = {}
        for name, shape, dt in [
            ("qt", [128, 2, NQC], FP16),
            ("kt", [128, 2, NK], FP16),
            ("wq", [128, 2, 256], FP16),
            ("wk", [128, 2, 256], FP16),
            ("wv", [128, 2, 256], FP16),
            ("wo", [128, 2, 256], FP16),
            ("bq", [128, 2], FP32),
            ("bk", [128, 2], FP32),
            ("bo", [128, 2], FP32),
            ("bvr", [128, 256], FP32),
            ("g0r", [128, 256], FP32),
            ("b0r", [128, 256], FP32),
            ("g1r", [128, 256], FP32),
            ("b1r", [128, 256], FP32),
        ]:
            t = singles.tile(shape, dt, tag=name)
            nc.sync.dma_start(t[:], d[name][:])
            sb[name] = t

        ident = singles.tile([128, 128], FP32, tag="ident")
        make_identity(nc, ident[:])
        ident16 = singles.tile([128, 128], FP16, tag="ident16")
        make_identity(nc, ident16[:])

        # persistent SBUF intermediates
        qp = singles.tile([128, 2, NQC], FP16, tag="qp")      # Qp^T
        kp = singles.tile([128, 2, NK], FP16, tag="kp")       # Kp^T (scaled)
        vpx = singles.tile([128, 16, H, 64], FP16, tag="vpx")  # [k, kt, h, V|1|0]
        o_nat = singles.tile([128, 8, 512], FP32, tag="onat")  # PV out natural
        odiv8 = singles.tile([128, 8, 256], FP32, tag="odiv8")
        oln8 = singles.tile([128, 8, 256], FP16, tag="oln8")   # LN0 normalized
        olngb = singles.tile([128, 8, 256], FP16, tag="olngb")  # * g0 + b0
        olnT = singles.tile([128, 2, NQC], FP16, tag="olnT")
        fcT = singles.tile([128, 2, NQC], FP16, tag="fcT")
        r3T = singles.tile([128, 2, NQC], FP16, tag="r3T")
        r3n8 = singles.tile([128, 8, 256], FP16, tag="r3n8")
        mv8a = singles.tile([128, 8, 2], FP32, tag="mv8a")
        mv8b = singles.tile([128, 8, 2], FP32, tag="mv8b")
        ve8a = singles.tile([128, 8], FP32, tag="ve8a")       # var+eps
        ve8b = singles.tile([128, 8], FP32, tag="ve8b")
        rsa = singles.tile([128, 8], FP32, tag="rsa")         # rstd stage 1
        rstd0 = singles.tile([128, 8], FP32, tag="rstd0")     # LN0 rstd
        rstd1 = singles.tile([128, 8], FP32, tag="rstd1")     # LN1 rstd

        nc.vector.memset(vpx[:], 0.0)
        nc.vector.memset(vpx[:, :, :, 32:33], 1.0)

        # ---- phase 1: projections ----
        with tc.tile_pool(name="prj_ps", bufs=2, space="PSUM") as prj_ps:
            # Qp^T[dv, q] and Kp^T[dv, k]; evict on ACT (Identity + bias)
            for (wname, bname, src, dst, ncols) in [
                ("wq", "bq", "qt", qp, NQC),
                ("wk", "bk", "kt", kp, NK),
            ]:
                for dvt in range(2):
                    for qcc in range(ncols // 512):
                        ps = prj_ps.tile([128, 512], FP32, tag="p512")
                        for o in range(2):
                            nc.tensor.matmul(
                                ps[:],
                                sb[wname][:, o, dvt * 128:(dvt + 1) * 128],
                                sb[src][:, o, qcc * 512:(qcc + 1) * 512],
                                start=(o == 0),
                                stop=(o == 1),
                            )
                        nc.scalar.activation(
                            out=dst[:, dvt, qcc * 512:(qcc + 1) * 512],
                            in_=ps[:],
                            func=mybir.ActivationFunctionType.Identity,
                            bias=sb[bname][:, dvt:dvt + 1],
                            scale=1.0,
                        )
            # Vp natural [k, dv] into 64-wide head blocks with ones column;
            # eviction + bv add fused on DVE.
            for kt_i in range(16):
                ps = prj_ps.tile([128, 256], FP32, tag="p256")
                for o in range(2):
                    nc.tensor.matmul(
                        ps[:],
                        sb["kt"][:, o, kt_i * 128:(kt_i + 1) * 128],
                        sb["wv"][:, o, :],
                        start=(o == 0),
                        stop=(o == 1),
                    )
                nc.vector.tensor_tensor(
                    out=vpx[:, kt_i, :, 0:32],
                    in0=ps[:].rearrange("p (h e) -> p h e", h=H),
                    in1=sb["bvr"][:].rearrange("p (h e) -> p h e", h=H),
                    op=mybir.AluOpType.add,
                )

        # ---- phase 2: attention + fused tail ----
        with (
            tc.tile_pool(name="sc_ps", bufs=3, space="PSUM") as sc_ps,
            tc.tile_pool(name="acc_ps", bufs=2, space="PSUM") as acc_ps,
            tc.tile_pool(name="et_sb", bufs=4) as et_sb,
            tc.tile_pool(name="ev_sb", bufs=6) as ev_sb,
        ):
            def attention_block(qc, j):
                """QK -> exp -> PV for heads (2j, 2j+1), q cols qc*512.."""
                pv = acc_ps.tile([128, 512], FP32, tag="pv")
                et_tiles = {}
                for step in range(18):
                    kt_i = step
                    if kt_i < 16:
                        sc = sc_ps.tile([128, 1024], FP32, tag="sc")
                        for hi in range(2):
                            h = 2 * j + hi
                            rp = (h % 4) * 32
                            chh = h // 4
                            nc.tensor.matmul(
                                sc[:, hi * 512:(hi + 1) * 512],
                                kp[rp:rp + 32, chh, kt_i * 128:(kt_i + 1) * 128],
                                qp[rp:rp + 32, chh, qc * 512:(qc + 1) * 512],
                                start=True,
                                stop=True,
                                tile_position=(rp, 0),
                            )
                        et = et_sb.tile([128, 1024], FP16, tag="et")
                        if kt_i in ACT_KT:
                            nc.scalar.activation(
                                out=et[:], in_=sc[:],
                                func=mybir.ActivationFunctionType.Exp,
                                scale=float(EXP_N),
                            )
                        else:
                            nc.vector._custom_dve(EXP64, out=et[:], in0=sc[:])
                        et_tiles[kt_i] = et
                    pkt = step - 2
                    if pkt >= 0:
                        et = et_tiles.pop(pkt)
                        for hi in range(2):
                            h = 2 * j + hi
                            nc.tensor.matmul(
                                pv[hi * 64:(hi + 1) * 64, :],
                                vpx[:, pkt, h, :],
                                et[:, hi * 512:(hi + 1) * 512],
                                start=(pkt == 0),
                                stop=(pkt == 15),
                                tile_position=(0, hi * 64),
                            )
                # evict PV accumulator, transpose to natural q rows
                pvs = ev_sb.tile([128, 512], FP32, tag="pvs")
                nc.vector.tensor_copy(out=pvs[:], in_=pv[:])
                trdst = sc_ps.tile([128, 512], FP32, tag="sc")
                for qs in range(4):
                    nc.tensor.transpose(
                        trdst[:, qs * 128:(qs + 1) * 128],
                        pvs[:, qs * 128:(qs + 1) * 128], ident[:])
                nc.scalar.copy(
                    out=o_nat[:, qc * 4:(qc + 1) * 4, j * 128:(j + 1) * 128],
                    in_=trdst[:].rearrange("p (q c) -> p q c", q=4),
                )

            def tail_a(qc):
                """softmax divide + LN0 stats for the 4 qsubs of qc."""
                for qs in range(4):
                    qsub = qc * 4 + qs
                    rd = small.tile([128, 8], FP32, tag="rd")
                    nc.vector.reciprocal_approx_fast(
                        out=rd[:], in_=o_nat[:, qsub, 32::64])
                    for h in range(H):
                        cb = (h // 2) * 128 + (h % 2) * 64
                        nc.gpsimd.tensor_scalar(
                            out=odiv8[:, qsub, h * 32:(h + 1) * 32],
                            in0=o_nat[:, qsub, cb:cb + 32],
                            scalar1=rd[:, h:h + 1],
                            scalar2=None,
                            op0=mybir.AluOpType.mult,
                        )
                    stats = small.tile([128, 6], FP32, tag="stats")
                    nc.vector.bn_stats(out=stats[:], in_=odiv8[:, qsub, :])
                    nc.vector.bn_aggr(out=mv8a[:, qsub, :], in_=stats[:])
                # rstd via fixed-seed Newton (no ACT table thrash)
                q0 = qc * 4
                nc.vector.tensor_scalar(
                    out=ve8a[:, q0:q0 + 4], in0=mv8a[:, q0:q0 + 4, 1],
                    scalar1=EPS, scalar2=None, op0=mybir.AluOpType.add,
                )
                y0 = RSQRT_SEED_LN0
                nc.vector._custom_dve(
                    NEWTON2, out=rsa[:, q0:q0 + 4], in0=ve8a[:, q0:q0 + 4],
                    s0=1.5 * y0, s1=0.5 * y0 ** 3, imm2=0.5,
                )
                nc.vector._custom_dve(
                    NEWTON1, out=rstd0[:, q0:q0 + 4],
                    in0=ve8a[:, q0:q0 + 4], in1=rsa[:, q0:q0 + 4], s0=0.5,
                )

            def tail_b(qc, qs):
                """LN0 apply + transpose + fc_o + residual + LN1 stats, 1 qsub."""
                qsub = qc * 4 + qs
                # LN0 apply: (x - m) * rstd -> fp16
                nc.vector.tensor_scalar(
                    out=oln8[:, qsub, :], in0=odiv8[:, qsub, :],
                    scalar1=mv8a[:, qsub, 0:1],
                    scalar2=rstd0[:, qsub:qsub + 1],
                    op0=mybir.AluOpType.subtract, op1=mybir.AluOpType.mult,
                )
                # * g0 + b0 on gpsimd
                nc.gpsimd.tensor_tensor(
                    out=olngb[:, qsub, :], in0=oln8[:, qsub, :],
                    in1=sb["g0r"][:], op=mybir.AluOpType.mult,
                )
                nc.gpsimd.tensor_tensor(
                    out=olngb[:, qsub, :], in0=olngb[:, qsub, :],
                    in1=sb["b0r"][:], op=mybir.AluOpType.add,
                )
                # transpose to [dv, q]
                trd = sc_ps.tile([128, 256], FP16, tag="sc")
                for dvt in range(2):
                    nc.tensor.transpose(
                        trd[:, dvt * 128:(dvt + 1) * 128],
                        olngb[:, qsub, dvt * 128:(dvt + 1) * 128], ident16[:])
                nc.scalar.copy(
                    out=olnT[:, :, qsub * 128:(qsub + 1) * 128],
                    in_=trd[:].rearrange("p (c q) -> p c q", c=2),
                )
                # fc_o for this qsub: [128dv x 128q] per dvt, contract 2 chunks
                fps = sc_ps.tile([128, 256], FP32, tag="sc")
                for dvt in range(2):
                    for o in range(2):
                        nc.tensor.matmul(
                            fps[:, dvt * 128:(dvt + 1) * 128],
                            sb["wo"][:, o, dvt * 128:(dvt + 1) * 128],
                            olnT[:, o, qsub * 128:(qsub + 1) * 128],
                            start=(o == 0),
                            stop=(o == 1),
                        )
                # relu(fc + bo) on ACT (same table as Exp)
                for dvt in range(2):
                    nc.scalar.activation(
                        out=fcT[:, dvt, qsub * 128:(qsub + 1) * 128],
                        in_=fps[:, dvt * 128:(dvt + 1) * 128],
                        func=mybir.ActivationFunctionType.Relu,
                        bias=sb["bo"][:, dvt:dvt + 1],
                        scale=1.0,
                    )
                # residual on gpsimd
                nc.gpsimd.tensor_tensor(
                    out=r3T[:, :, qsub * 128:(qsub + 1) * 128],
                    in0=olnT[:, :, qsub * 128:(qsub + 1) * 128],
                    in1=fcT[:, :, qsub * 128:(qsub + 1) * 128],
                    op=mybir.AluOpType.add,
                )
                # transpose back to natural
                trn = sc_ps.tile([128, 256], FP16, tag="sc")
                for dvt in range(2):
                    nc.tensor.transpose(
                        trn[:, dvt * 128:(dvt + 1) * 128],
                        r3T[:, dvt, qsub * 128:(qsub + 1) * 128], ident16[:])
                nc.vector.tensor_copy(out=r3n8[:, qsub, :], in_=trn[:])
                stats = small.tile([128, 6], FP32, tag="stats")
                nc.vector.bn_stats(out=stats[:], in_=r3n8[:, qsub, :])
                nc.vector.bn_aggr(out=mv8b[:, qsub, :], in_=stats[:])

            def tail_c(qc):
                """LN1 rstd + apply + g1/b1 + store for the 4 qsubs of qc."""
                q0 = qc * 4
                nc.vector.tensor_scalar(
                    out=ve8b[:, q0:q0 + 4], in0=mv8b[:, q0:q0 + 4, 1],
                    scalar1=EPS, scalar2=None, op0=mybir.AluOpType.add,
                )
                y0 = RSQRT_SEED_LN1
                nc.vector._custom_dve(
                    NEWTON2, out=rsa[:, q0:q0 + 4], in0=ve8b[:, q0:q0 + 4],
                    s0=1.5 * y0, s1=0.5 * y0 ** 3, imm2=0.5,
                )
                nc.vector._custom_dve(
                    NEWTON1, out=rstd1[:, q0:q0 + 4],
                    in0=ve8b[:, q0:q0 + 4], in1=rsa[:, q0:q0 + 4], s0=0.5,
                )
                for qs in range(4):
                    qsub = q0 + qs
                    xn = ev_sb.tile([128, 256], FP32, tag="xn")
                    nc.vector.tensor_scalar(
                        out=xn[:], in0=r3n8[:, qsub, :],
                        scalar1=mv8b[:, qsub, 0:1],
                        scalar2=rstd1[:, qsub:qsub + 1],
                        op0=mybir.AluOpType.subtract, op1=mybir.AluOpType.mult,
                    )
                    fin = ev_sb.tile([128, 256], FP32, tag="fin")
                    nc.gpsimd.tensor_tensor(
                        out=fin[:], in0=xn[:], in1=sb["g1r"][:],
                        op=mybir.AluOpType.mult,
                    )
                    nc.gpsimd.tensor_tensor(
                        out=fin[:], in0=fin[:], in1=sb["b1r"][:],
                        op=mybir.AluOpType.add,
                    )
                    nc.sync.dma_start(
                        out_dram[qsub * 128:(qsub + 1) * 128, :], fin[:])

            # qc0 attention
            for j in range(4):
                attention_block(0, j)
            tail_a(0)
            # qc1 attention with qc0's per-qsub tails interleaved
            for j in range(4):
                attention_block(1, j)
                tail_b(0, j)
            tail_c(0)
            tail_a(1)
            for qs in range(4):
                tail_b(1, qs)
            tail_c(1)


_NC = None


def _get_nc():
    global _NC
    if _NC is None:
        _NC = _build()
    return _NC


def _chunk_major(v):
    # [256] channel vector -> [128, 2] where [p, o] = v[o*128+p]
    return np.ascontiguousarray(v.reshape(2, 128).T.astype(np.float32))


def _prep_inputs(Q, K, Wq, bq, Wk, bk, Wv, bv, Wo, bo, g0, b0, g1, b1):
    def t_chunks(m, dt):
        # [256, n] -> [128, 2, n]: row d = o*128+p goes to [p, o, :]
        return np.ascontiguousarray(
            m.reshape(2, 128, m.shape[1]).transpose(1, 0, 2).astype(dt)
        )

    kscale = SCALE / EXP_N  # fold score scale + exp prescale into Wk
    wq_t = t_chunks(Wq.T, np.float16)
    wk_t = t_chunks((Wk * kscale).T, np.float16)
    wv_t = t_chunks(Wv.T, np.float16)
    wo_t = t_chunks(Wo.T, np.float16)
    rep = lambda v: np.ascontiguousarray(
        np.broadcast_to(v.astype(np.float32), (128, 256))
    )
    common = {
        "wq": wq_t, "wk": wk_t, "wv": wv_t, "wo": wo_t,
        "bq": _chunk_major(bq), "bk": _chunk_major(bk * kscale),
        "bo": _chunk_major(bo),
        "bvr": rep(bv), "g0r": rep(g0), "b0r": rep(b0),
        "g1r": rep(g1), "b1r": rep(b1),
    }
    in_maps = []
    for c in range(8):
        b, qh = c // 2, c % 2
        qt = t_chunks(Q[b, qh * NQC:(qh + 1) * NQC, :].T, np.float16)
        kt = t_chunks(K[b].T, np.float16)
        in_maps.append({"qt": qt, "kt": kt, **common})
    return in_maps


def _run(inputs, trace=False):
    nc = _get_nc()
    in_maps = _prep_inputs(**inputs)
    res = bass_utils.run_bass_kernel_spmd(
        nc, in_maps, core_ids=list(range(8)), trace=trace
    )
    out = np.empty((B, NQ, DV), np.float32)
    for c in range(8):
        b, qh = c // 2, c % 2
        out[b, qh * NQC:(qh + 1) * NQC, :] = res.results[c]["out"]
    return out, res


def kernel(**inputs):
    inputs = {k: np.asarray(v) for k, v in inputs.items()}
    out, _ = _run(inputs, trace=False)
    return out


# revision 14
# speedup vs baseline: 1.2715x; 1.2715x over previous
"""Trainium2 Bass kernel for nn_MAB: MHA block (B=4, N=2048, D=256, H=8) on 8 cores.

Sharding: 8 shards = (batch b, query-half) pairs. Each core computes the full
attention + LN/FFN tail for its 1024 query rows against all 2048 keys of its
batch. All gathering happens on host; no collectives.

v2 structure (vs baseline):
- softmax exp is split across TWO engines: ACT (exact exp, scale=64) and DVE
  (custom fused op computing (1+x/64)^64 ~= exp(x), 7 ALU blocks, 1 elem/cyc).
  The x/64 pre-scale is folded into Wk on the host.
- PE stream is software-pipelined: QK(kt) is emitted two steps before PV(kt)
  so the tensor engine never waits on the exp engines (sc PSUM pool bufs=3).
- PSUM evictions with per-partition bias ride the Scalar engine (Identity /
  Relu / Copy live in the same ACT table as Exp -> no table reloads at all).
- LN rstd uses fixed-seed Newton iterations in custom DVE ops (no Sqrt table).
- LN gamma/beta applies, softmax divides and the residual add run on the
  otherwise-idle GpSimd engine.
- fc_o runs in fp16 (Wo cast on host); LN0/fc/residual chain keeps fp16
  tensors so PE transposes cost 1 cycle/row.
"""

import numpy as np

import concourse.bass as bass
import concourse.tile as tile
from concourse import bacc, mybir
from concourse import bass_utils
from concourse import dve_ops
from concourse.dve_spec import (
    C0, C1, C2, Spec, Src0, Src1, One, sq, lower, spec_leaves,
)
from concourse.dve_uop import DveOpSpec
from concourse.masks import make_identity

B, NQ, NK, DV, H = 4, 2048, 2048, 256, 8
HD = DV // H  # 32
NQC = 1024  # q rows per core
SCALE = 1.0 / np.sqrt(HD)
EXP_N = 64.0  # (1+x/N)^N exp approximation on DVE
EPS = 1e-5
FP16 = mybir.dt.float16
FP32 = mybir.dt.float32

# kt tiles (of 16 per head-pair) whose exp runs on ACT; rest on DVE.
ACT_KT = frozenset({0, 1, 2, 4, 6, 8, 10, 12, 14})

# Newton rsqrt seeds (only need to be within ~50% of 1/sqrt(var+eps)).
RSQRT_SEED_LN0 = 130.0   # var(o/denom) ~ 5e-5 (+eps 1e-5)
RSQRT_SEED_LN1 = 0.95    # var(ln0 + relu(fc)) ~ 1.1


def _register():
    """Register custom DVE ops (exp approx + Newton rsqrt); shas are
    computed at import so they always match the lowering."""
    ver = "v3"  # TRN2
    made = {}

    def add(name, spec):
        existing = {o.name: o for o in dve_ops.OPS}
        if name in existing:
            made[name] = existing[name]
            return
        row = max(dve_ops._SUB_OPCODE_FOR_NAME.values()) + 1
        assert row < 0x20
        dve_ops._SUB_OPCODE_FOR_NAME[name] = row
        uops = lower(spec, ver=ver)
        sha = DveOpSpec(
            name=name, opcode=row, uops=uops, rd1_en=Src1 in spec_leaves(spec)
        ).sha(ver)
        op = dve_ops.DveOp(name, spec, subdim=False, uops_sha={ver: sha})
        dve_ops.OPS.append(op)
        dve_ops.CUSTOM_DVE_SPECS[name] = spec
        made[name] = op

    # out = (1 + x)^64  (input pre-scaled by 1/64; approximates exp(64x))
    add("EXP_POW64_ANT", Spec(
        body=sq(sq(sq(sq(sq(sq(One + Src0)))))),
        reference=lambda in0, in1, s0, s1, imm2:
            ((1.0 + in0.astype(np.float64)) ** 64).astype(np.float32),
    ))

    # Two Newton rsqrt iterations from a compile-time seed y0:
    #   y1 = C0 - v*C1                 (C0 = 1.5*y0, C1 = 0.5*y0^3)
    #   out = y1*((1+C2) - (v*y1^2)*C2)  (C2 = 0.5)
    y1 = C0 - Src0 * C1
    add("RSQRT_NEWTON2_ANT", Spec(
        body=y1 * ((One + C2) - (Src0 * sq(y1)) * C2),
        reference=lambda in0, in1, s0, s1, imm2: (
            lambda v, yy: (yy * (1.5 - 0.5 * v * yy * yy))
        )(in0.astype(np.float64),
          s0 - in0.astype(np.float64) * s1).astype(np.float32),
    ))

    # One more Newton step: out = y*((1+C0) - (v*y^2)*C0)  (C0 = 0.5)
    add("RSQRT_NEWTON1_ANT", Spec(
        body=Src1 * ((One + C0) - (Src0 * sq(Src1)) * C0),
        reference=lambda in0, in1, s0, s1, imm2:
            (in1.astype(np.float64) * (1.5 - 0.5 * in0.astype(np.float64)
             * in1.astype(np.float64) ** 2)).astype(np.float32),
    ))
    return made


_DVE_OPS = _register()
EXP64 = _DVE_OPS["EXP_POW64_ANT"]
NEWTON2 = _DVE_OPS["RSQRT_NEWTON2_ANT"]
NEWTON1 = _DVE_OPS["RSQRT_NEWTON1_ANT"]


def _build():
    nc = bacc.Bacc(
        "TRN2",
        target_bir_lowering=False,
        debug=False,
        enable_asserts=False,
        num_devices=1,
    )
    d = {}
    ins = [
        ("qt", [128, 2, NQC], FP16),       # Q-shard^T  [dq(part), dq-chunk, q]
        ("kt", [128, 2, NK], FP16),        # K^T        [dq(part), dq-chunk, k]
        ("wq", [128, 2, 256], FP16),       # Wq^T       [dq(part), dq-chunk, dv]
        ("wk", [128, 2, 256], FP16),       # pre-scaled by SCALE/EXP_N
        ("wv", [128, 2, 256], FP16),
        ("wo", [128, 2, 256], FP16),       # Wo^T
        ("bq", [128, 2], FP32),            # per-dv-channel biases, chunk-major
        ("bk", [128, 2], FP32),
        ("bo", [128, 2], FP32),
        ("bvr", [128, 256], FP32),         # bv replicated over partitions
        ("g0c", [128, 2], FP32),           # g0/b0 chunk-major (ACT scale/bias)
        ("b0c", [128, 2], FP32),
        ("g1r", [128, 256], FP32),
        ("b1r", [128, 256], FP32),
    ]
    for name, shape, dt in ins:
        d[name] = nc.dram_tensor(name, shape, dt, kind="ExternalInput").ap()
    out_dram = nc.dram_tensor("out", [NQC, 256], FP32, kind="ExternalOutput").ap()

    with tile.TileContext(nc) as tc:
        _kernel_body(tc, d, out_dram)
    nc.compile()
    return nc


def _kernel_body(tc, d, out_dram):
    nc = tc.nc
    from contextlib import ExitStack

    ctx = ExitStack()
    with ctx:
        singles = ctx.enter_context(tc.tile_pool(name="singles", bufs=1))
        small = ctx.enter_context(tc.tile_pool(name="small", bufs=8))

        # ---- load constants / inputs to SBUF ----
        sb = {}
        for name, shape, dt in [
            ("qt", [128, 2, NQC], FP16),
            ("kt", [128, 2, NK], FP16),
            ("wq", [128, 2, 256], FP16),
            ("wk", [128, 2, 256], FP16),
            ("wv", [128, 2, 256], FP16),
            ("wo", [128, 2, 256], FP16),
            ("bq", [128, 2], FP32),
            ("bk", [128, 2], FP32),
            ("bo", [128, 2], FP32),
            ("bvr", [128, 256], FP32),
            ("g0c", [128, 2], FP32),
            ("b0c", [128, 2], FP32),
            ("g1r", [128, 256], FP32),
            ("b1r", [128, 256], FP32),
        ]:
            t = singles.tile(shape, dt, tag=name)
            nc.sync.dma_start(t[:], d[name][:])
            sb[name] = t

        ident = singles.tile([128, 128], FP32, tag="ident")
        make_identity(nc, ident[:])
        ident16 = singles.tile([128, 128], FP16, tag="ident16")
        make_identity(nc, ident16[:])

        # persistent SBUF intermediates
        qp = singles.tile([128, 2, NQC], FP16, tag="qp")      # Qp^T
        kp = singles.tile([128, 2, NK], FP16, tag="kp")       # Kp^T (scaled)
        vpx = singles.tile([128, 16, H, 64], FP16, tag="vpx")  # [k, kt, h, V|1|0]
        o_nat = singles.tile([128, 8, 512], FP32, tag="onat")  # PV out natural
        odiv8 = singles.tile([128, 8, 256], FP32, tag="odiv8")
        oln8 = singles.tile([128, 8, 256], FP16, tag="oln8")   # LN0 normalized
        olnT = singles.tile([128, 2, NQC], FP16, tag="olnT")
        fcT = singles.tile([128, 2, NQC], FP16, tag="fcT")
        r3T = singles.tile([128, 2, NQC], FP16, tag="r3T")
        r3n8 = singles.tile([128, 8, 256], FP16, tag="r3n8")
        mv8a = singles.tile([128, 8, 2], FP32, tag="mv8a")
        mv8b = singles.tile([128, 8, 2], FP32, tag="mv8b")
        ve8a = singles.tile([128, 8], FP32, tag="ve8a")       # var+eps
        ve8b = singles.tile([128, 8], FP32, tag="ve8b")
        rsa = singles.tile([128, 8], FP32, tag="rsa")         # rstd stage 1
        rstd0 = singles.tile([128, 8], FP32, tag="rstd0")     # LN0 rstd
        rstd1 = singles.tile([128, 8], FP32, tag="rstd1")     # LN1 rstd

        nc.vector.memset(vpx[:], 0.0)
        nc.vector.memset(vpx[:, :, :, 32:33], 1.0)

        # ---- phase 1: projections ----
        with tc.tile_pool(name="prj_ps", bufs=2, space="PSUM") as prj_ps:
            # Qp^T[dv, q] and Kp^T[dv, k]; evict on ACT (Identity + bias)
            for (wname, bname, src, dst, ncols) in [
                ("wq", "bq", "qt", qp, NQC),
                ("wk", "bk", "kt", kp, NK),
            ]:
                for dvt in range(2):
                    for qcc in range(ncols // 512):
                        ps = prj_ps.tile([128, 512], FP32, tag="p512")
                        for o in range(2):
                            nc.tensor.matmul(
                                ps[:],
                                sb[wname][:, o, dvt * 128:(dvt + 1) * 128],
                                sb[src][:, o, qcc * 512:(qcc + 1) * 512],
                                start=(o == 0),
                                stop=(o == 1),
                            )
                        nc.scalar.activation(
                            out=dst[:, dvt, qcc * 512:(qcc + 1) * 512],
                            in_=ps[:],
                            func=mybir.ActivationFunctionType.Identity,
                            bias=sb[bname][:, dvt:dvt + 1],
                            scale=1.0,
                        )
            # Vp natural [k, dv] into 64-wide head blocks with ones column;
            # eviction + bv add fused on DVE.
            for kt_i in range(16):
                ps = prj_ps.tile([128, 256], FP32, tag="p256")
                for o in range(2):
                    nc.tensor.matmul(
                        ps[:],
                        sb["kt"][:, o, kt_i * 128:(kt_i + 1) * 128],
                        sb["wv"][:, o, :],
                        start=(o == 0),
                        stop=(o == 1),
                    )
                nc.vector.tensor_tensor(
                    out=vpx[:, kt_i, :, 0:32],
                    in0=ps[:].rearrange("p (h e) -> p h e", h=H),
                    in1=sb["bvr"][:].rearrange("p (h e) -> p h e", h=H),
                    op=mybir.AluOpType.add,
                )

        # ---- phase 2: attention + fused tail ----
        with (
            tc.tile_pool(name="sc_ps", bufs=3, space="PSUM") as sc_ps,
            tc.tile_pool(name="acc_ps", bufs=2, space="PSUM") as acc_ps,
            tc.tile_pool(name="et_sb", bufs=4) as et_sb,
            tc.tile_pool(name="ev_sb", bufs=6) as ev_sb,
        ):
            def attention_block(qc, j, n_act):
                """QK -> exp -> PV for heads (2j, 2j+1), q cols qc*512.."""
                pv = acc_ps.tile([128, 512], FP32, tag="pv")
                et_tiles = {}
                for step in range(18):
                    kt_i = step
                    if kt_i < 16:
                        sc = sc_ps.tile([128, 1024], FP32, tag="sc")
                        for hi in range(2):
                            h = 2 * j + hi
                            rp = (h % 4) * 32
                            chh = h // 4
                            nc.tensor.matmul(
                                sc[:, hi * 512:(hi + 1) * 512],
                                kp[rp:rp + 32, chh, kt_i * 128:(kt_i + 1) * 128],
                                qp[rp:rp + 32, chh, qc * 512:(qc + 1) * 512],
                                start=True,
                                stop=True,
                                tile_position=(rp, 0),
                            )
                        et = et_sb.tile([128, 1024], FP16, tag="et")
                        if (kt_i * n_act) % 16 < n_act:
                            nc.scalar.activation(
                                out=et[:], in_=sc[:],
                                func=mybir.ActivationFunctionType.Exp,
                                scale=float(EXP_N),
                            )
                        else:
                            nc.vector._custom_dve(EXP64, out=et[:], in0=sc[:])
                        et_tiles[kt_i] = et
                    pkt = step - 2
                    if pkt >= 0:
                        et = et_tiles.pop(pkt)
                        for hi in range(2):
                            h = 2 * j + hi
                            nc.tensor.matmul(
                                pv[hi * 64:(hi + 1) * 64, :],
                                vpx[:, pkt, h, :],
                                et[:, hi * 512:(hi + 1) * 512],
                                start=(pkt == 0),
                                stop=(pkt == 15),
                                tile_position=(0, hi * 64),
                            )
                # evict PV accumulator, transpose to natural q rows
                pvs = ev_sb.tile([128, 512], FP32, tag="pvs")
                nc.vector.tensor_copy(out=pvs[:], in_=pv[:])
                trdst = sc_ps.tile([128, 512], FP32, tag="sc")
                for qs in range(4):
                    nc.tensor.transpose(
                        trdst[:, qs * 128:(qs + 1) * 128],
                        pvs[:, qs * 128:(qs + 1) * 128], ident[:])
                nc.scalar.copy(
                    out=o_nat[:, qc * 4:(qc + 1) * 4, j * 128:(j + 1) * 128],
                    in_=trdst[:].rearrange("p (q c) -> p q c", q=4),
                )

            def stage_a(qsub):
                """softmax divide + LN0 stats + rstd + LN0 apply (DVE only)."""
                rd = small.tile([128, 8], FP32, tag="rd")
                nc.vector.reciprocal_approx_fast(
                    out=rd[:], in_=o_nat[:, qsub, 32::64])
                for h in range(H):
                    cb = (h // 2) * 128 + (h % 2) * 64
                    nc.vector.tensor_scalar(
                        out=odiv8[:, qsub, h * 32:(h + 1) * 32],
                        in0=o_nat[:, qsub, cb:cb + 32],
                        scalar1=rd[:, h:h + 1],
                        scalar2=None,
                        op0=mybir.AluOpType.mult,
                    )
                stats = small.tile([128, 6], FP32, tag="stats")
                nc.vector.bn_stats(out=stats[:], in_=odiv8[:, qsub, :])
                nc.vector.bn_aggr(out=mv8a[:, qsub, :], in_=stats[:])
                # rstd via fixed-seed Newton (no ACT table thrash)
                nc.vector.tensor_scalar(
                    out=ve8a[:, qsub:qsub + 1], in0=mv8a[:, qsub, 1:2],
                    scalar1=EPS, scalar2=None, op0=mybir.AluOpType.add,
                )
                y0 = RSQRT_SEED_LN0
                nc.vector._custom_dve(
                    NEWTON2, out=rsa[:, qsub:qsub + 1],
                    in0=ve8a[:, qsub:qsub + 1],
                    s0=1.5 * y0, s1=0.5 * y0 ** 3, imm2=0.5,
                )
                nc.vector._custom_dve(
                    NEWTON1, out=rstd0[:, qsub:qsub + 1],
                    in0=ve8a[:, qsub:qsub + 1], in1=rsa[:, qsub:qsub + 1],
                    s0=0.5,
                )
                # LN0 apply: (x - m) * rstd -> fp16 (gamma/beta folded later)
                nc.vector.tensor_scalar(
                    out=oln8[:, qsub, :], in0=odiv8[:, qsub, :],
                    scalar1=mv8a[:, qsub, 0:1],
                    scalar2=rstd0[:, qsub:qsub + 1],
                    op0=mybir.AluOpType.subtract, op1=mybir.AluOpType.mult,
                )

            def stage_b(qsub):
                """transpose LN0; fold g0/b0 into the ACT eviction."""
                trd = sc_ps.tile([128, 256], FP16, tag="sc")
                for dvt in range(2):
                    nc.tensor.transpose(
                        trd[:, dvt * 128:(dvt + 1) * 128],
                        oln8[:, qsub, dvt * 128:(dvt + 1) * 128], ident16[:])
                for dvt in range(2):
                    nc.scalar.activation(
                        out=olnT[:, dvt, qsub * 128:(qsub + 1) * 128],
                        in_=trd[:, dvt * 128:(dvt + 1) * 128],
                        func=mybir.ActivationFunctionType.Identity,
                        bias=sb["b0c"][:, dvt:dvt + 1],
                        scale=sb["g0c"][:, dvt:dvt + 1],
                    )

            def stage_c(qsub):
                """fc_o matmul + relu eviction + residual."""
                fps = sc_ps.tile([128, 256], FP32, tag="sc")
                for dvt in range(2):
                    for o in range(2):
                        nc.tensor.matmul(
                            fps[:, dvt * 128:(dvt + 1) * 128],
                            sb["wo"][:, o, dvt * 128:(dvt + 1) * 128],
                            olnT[:, o, qsub * 128:(qsub + 1) * 128],
                            start=(o == 0),
                            stop=(o == 1),
                        )
                for dvt in range(2):
                    nc.scalar.activation(
                        out=fcT[:, dvt, qsub * 128:(qsub + 1) * 128],
                        in_=fps[:, dvt * 128:(dvt + 1) * 128],
                        func=mybir.ActivationFunctionType.Relu,
                        bias=sb["bo"][:, dvt:dvt + 1],
                        scale=1.0,
                    )
                nc.vector.tensor_tensor(
                    out=r3T[:, :, qsub * 128:(qsub + 1) * 128],
                    in0=olnT[:, :, qsub * 128:(qsub + 1) * 128],
                    in1=fcT[:, :, qsub * 128:(qsub + 1) * 128],
                    op=mybir.AluOpType.add,
                )

            def stage_d(qsub):
                """transpose back + LN1 stats + rstd + apply + g1/b1 + store."""
                trn = sc_ps.tile([128, 256], FP16, tag="sc")
                for dvt in range(2):
                    nc.tensor.transpose(
                        trn[:, dvt * 128:(dvt + 1) * 128],
                        r3T[:, dvt, qsub * 128:(qsub + 1) * 128], ident16[:])
                nc.vector.tensor_copy(out=r3n8[:, qsub, :], in_=trn[:])
                stats = small.tile([128, 6], FP32, tag="stats")
                nc.vector.bn_stats(out=stats[:], in_=r3n8[:, qsub, :])
                nc.vector.bn_aggr(out=mv8b[:, qsub, :], in_=stats[:])
                nc.vector.tensor_scalar(
                    out=ve8b[:, qsub:qsub + 1], in0=mv8b[:, qsub, 1:2],
                    scalar1=EPS, scalar2=None, op0=mybir.AluOpType.add,
                )
                y0 = RSQRT_SEED_LN1
                nc.vector._custom_dve(
                    NEWTON2, out=rsa[:, qsub:qsub + 1],
                    in0=ve8b[:, qsub:qsub + 1],
                    s0=1.5 * y0, s1=0.5 * y0 ** 3, imm2=0.5,
                )
                nc.vector._custom_dve(
                    NEWTON1, out=rstd1[:, qsub:qsub + 1],
                    in0=ve8b[:, qsub:qsub + 1], in1=rsa[:, qsub:qsub + 1],
                    s0=0.5,
                )
                xn = ev_sb.tile([128, 256], FP32, tag="xn")
                nc.vector.tensor_scalar(
                    out=xn[:], in0=r3n8[:, qsub, :],
                    scalar1=mv8b[:, qsub, 0:1],
                    scalar2=rstd1[:, qsub:qsub + 1],
                    op0=mybir.AluOpType.subtract, op1=mybir.AluOpType.mult,
                )
                fin = ev_sb.tile([128, 256], FP32, tag="fin")
                nc.vector.tensor_tensor(
                    out=fin[:], in0=xn[:], in1=sb["g1r"][:],
                    op=mybir.AluOpType.mult,
                )
                nc.vector.tensor_tensor(
                    out=fin[:], in0=fin[:], in1=sb["b1r"][:],
                    op=mybir.AluOpType.add,
                )
                nc.sync.dma_start(
                    out_dram[qsub * 128:(qsub + 1) * 128, :], fin[:])

            # qc0 attention: no tail overlap -> balanced exp split
            for j in range(4):
                attention_block(0, j, n_act=9)
            # qc1 attention with qc0's tail stages pipelined between blocks;
            # qc1 blocks lean harder on ACT since DVE carries the tail.
            stage_a(0)
            attention_block(1, 0, n_act=11)
            stage_a(1); stage_b(0)
            attention_block(1, 1, n_act=11)
            stage_a(2); stage_b(1); stage_c(0)
            attention_block(1, 2, n_act=11)
            stage_a(3); stage_b(2); stage_c(1); stage_d(0)
            attention_block(1, 3, n_act=11)
            stage_b(3); stage_c(2); stage_d(1)
            # qc1 tails (exposed end): pipeline the 4 qsubs across stages
            stage_a(4); stage_c(3); stage_d(2)
            stage_a(5); stage_b(4); stage_d(3)
            stage_a(6); stage_b(5); stage_c(4)
            stage_a(7); stage_b(6); stage_c(5); stage_d(4)
            stage_b(7); stage_c(6); stage_d(5)
            stage_c(7); stage_d(6)
            stage_d(7)


_NC = None


def _get_nc():
    global _NC
    if _NC is None:
        _NC = _build()
    return _NC


def _chunk_major(v):
    # [256] channel vector -> [128, 2] where [p, o] = v[o*128+p]
    return np.ascontiguousarray(v.reshape(2, 128).T.astype(np.float32))


def _prep_inputs(Q, K, Wq, bq, Wk, bk, Wv, bv, Wo, bo, g0, b0, g1, b1):
    def t_chunks(m, dt):
        # [256, n] -> [128, 2, n]: row d = o*128+p goes to [p, o, :]
        return np.ascontiguousarray(
            m.reshape(2, 128, m.shape[1]).transpose(1, 0, 2).astype(dt)
        )

    kscale = SCALE / EXP_N  # fold score scale + exp prescale into Wk
    wq_t = t_chunks(Wq.T, np.float16)
    wk_t = t_chunks((Wk * kscale).T, np.float16)
    wv_t = t_chunks(Wv.T, np.float16)
    wo_t = t_chunks(Wo.T, np.float16)
    rep = lambda v: np.ascontiguousarray(
        np.broadcast_to(v.astype(np.float32), (128, 256))
    )
    common = {
        "wq": wq_t, "wk": wk_t, "wv": wv_t, "wo": wo_t,
        "bq": _chunk_major(bq), "bk": _chunk_major(bk * kscale),
        "bo": _chunk_major(bo),
        "bvr": rep(bv), "g0c": _chunk_major(g0), "b0c": _chunk_major(b0),
        "g1r": rep(g1), "b1r": rep(b1),
    }
    in_maps = []
    for c in range(8):
        b, qh = c // 2, c % 2
        qt = t_chunks(Q[b, qh * NQC:(qh + 1) * NQC, :].T, np.float16)
        kt = t_chunks(K[b].T, np.float16)
        in_maps.append({"qt": qt, "kt": kt, **common})
    return in_maps


def _run(inputs, trace=False):
    nc = _get_nc()
    in_maps = _prep_inputs(**inputs)
    res = bass_utils.run_bass_kernel_spmd(
        nc, in_maps, core_ids=list(range(8)), trace=trace
    )
    out = np.empty((B, NQ, DV), np.float32)
    for c in range(8):
        b, qh = c // 2, c % 2
        out[b, qh * NQC:(qh + 1) * NQC, :] = res.results[c]["out"]
    return out, res


def kernel(**inputs):
    inputs = {k: np.asarray(v) for k, v in inputs.items()}
    out, _ = _run(inputs, trace=False)
    return out
